# revision 69
# baseline (speedup 1.0000x reference)
"""Multi-head attention TRN2 kernel (nn_Attention_48859547959768).

Head-parallel tensor parallelism across 8 NeuronCores: each core computes
2 of the 16 heads end-to-end (column-parallel QKV projection, attention,
row-parallel output projection) and returns a partial [B,S,DIM] output;
the host sums the 8 partials and adds the output bias.

v2 restructure (vs baseline): the kernel is organized as an explicit
8-slot software pipeline around the ScalarE exp stream, which is the
hard bottleneck (~16.8M exps/core, ScalarE-only).

  - Attention is split into S-blocks (scores matmuls + exp) and P-blocks
    (PV accumulation + drain). Units are (b, h, span-pair); emission is
    S(j) ; P(j-1) ; side-work(j) so ScalarE always has the next S-block's
    scores available and never waits on PV.
  - exp writes bf16 e-tiles into a 18-deep ring so a full span-pair of
    exps (16 tiles) can be outstanding while PV of the previous pair runs.
  - Projections/weights/X all bf16: halves X DMA and PE weight-load time.
  - proj(b1), V-transpose dance(b1), outproj(b0), and the per-(b,h,spp)
    softmax normalization are interleaved into pipeline slots as side
    blocks, sized to fit the PE slack under each 20us exp window.
  - Normalization: reciprocal of the denominator row ([1,S] on DVE),
    gpsimd partition_broadcast, one tensor_mul -- no DMA transposes
    (the baseline's 4-byte-descriptor DMA dance saturated the DMA rings).
  - Output stored bf16 (halves the tail store); final outproj drains
    split between DVE and ScalarE (idle after the last exp).
"""

import numpy as np

B, S, DIM = 2, 2048, 1024
H, D = 16, 64
N_CORES = 8
HPC = H // N_CORES  # heads per core = 2
DHC = HPC * D       # per-core head-dim slice = 128
KT = DIM // 128     # contraction tiles for projections = 8
NSPAN = S // 512    # q spans = 4
NSPP = NSPAN // 2   # span pairs = 2
NCHUNK = S // 128   # 128-token chunks = 16

_cached = {}


def _build():
    import concourse.mybir as mybir
    from concourse import bacc
    from concourse.masks import make_identity
    from concourse.tile import TileContext

    f32 = mybir.dt.float32
    bf16 = mybir.dt.bfloat16
    Exp = mybir.ActivationFunctionType.Exp
    Copy = mybir.ActivationFunctionType.Copy

    nc = bacc.Bacc("TRN2", target_bir_lowering=False)

    xt = nc.dram_tensor("xt", [B, DIM, S], bf16, kind="ExternalInput").ap()
    pen = nc.dram_tensor("pen", [B, 128, NCHUNK], f32, kind="ExternalInput").ap()
    wq = nc.dram_tensor("wq", [DIM, DHC], bf16, kind="ExternalInput").ap()
    wk = nc.dram_tensor("wk", [DIM, DHC], bf16, kind="ExternalInput").ap()
    wv = nc.dram_tensor("wv", [DIM, DHC], bf16, kind="ExternalInput").ap()
    wo = nc.dram_tensor("wo", [DHC, DIM], bf16, kind="ExternalInput").ap()
    bqd = nc.dram_tensor("bq", [DHC, 1], f32, kind="ExternalInput").ap()
    bkd = nc.dram_tensor("bk", [DHC, 1], f32, kind="ExternalInput").ap()
    bvd = nc.dram_tensor("bv", [DHC, 1], f32, kind="ExternalInput").ap()
    out = nc.dram_tensor("out", [B, S, DIM], bf16, kind="ExternalOutput").ap()

    with TileContext(nc) as tc:
        from contextlib import ExitStack

        with ExitStack() as ctx:
            const = ctx.enter_context(tc.tile_pool(name="const", bufs=1))
            xtp = ctx.enter_context(tc.tile_pool(name="xtp", bufs=KT))
            persist = ctx.enter_context(tc.tile_pool(name="persist", bufs=2))
            ctxp = ctx.enter_context(tc.tile_pool(name="ctxp", bufs=2))
            epool = ctx.enter_context(tc.tile_pool(name="epool", bufs=18))
            work = ctx.enter_context(tc.tile_pool(name="work", bufs=3))
            ps_sc = ctx.enter_context(tc.tile_pool(name="ps_sc", bufs=2, space="PSUM"))
            ps_pc = ctx.enter_context(tc.tile_pool(name="ps_pc", bufs=2, space="PSUM"))
            ps_po = ctx.enter_context(tc.tile_pool(name="ps_po", bufs=2, space="PSUM"))

            ident = const.tile([128, 128], bf16)
            make_identity(nc, ident)
            # PE warmup: keep TensorE busy through the initial DMA load so
            # the PE p-state ramps before the projections start.
            wps = ps_po.tile([128, 512], f32, tag="po", name="wps")
            for _ in range(84):
                nc.tensor.matmul(wps[:, 0:128], ident, ident,
                                 start=True, stop=True)
            # issue K/Q weights before X (they gate the first projections);
            # V and output weights can trail the X load.
            wq_sb = const.tile([128, KT, DHC], bf16)
            wk_sb = const.tile([128, KT, DHC], bf16)
            wv_sb = const.tile([128, KT, DHC], bf16)
            wo_sb = const.tile([128, DIM], bf16)
            bq_sb = const.tile([128, 1], f32)
            bk_sb = const.tile([128, 1], f32)
            bv_sb = const.tile([128, 1], f32)
            nc.sync.dma_start(out=wq_sb, in_=wq.rearrange("(kt p) m -> p kt m", p=128))
            nc.sync.dma_start(out=wk_sb, in_=wk.rearrange("(kt p) m -> p kt m", p=128))
            nc.sync.dma_start(out=wv_sb, in_=wv.rearrange("(kt p) m -> p kt m", p=128))
            nc.sync.dma_start(out=wo_sb, in_=wo)
            nc.sync.dma_start(out=bq_sb, in_=bqd)
            nc.sync.dma_start(out=bk_sb, in_=bkd)
            nc.sync.dma_start(out=bv_sb, in_=bvd)

            st = [dict() for _ in range(B)]

            # ---------------- phase builders ----------------
            def phase_load(b):
                # h-half-major issue order: the first-half token columns of
                # every kt tile land first so K/Q(spans 0,1) can start while
                # the second halves stream in.
                with nc.named_scope(f"load{b}"):
                    xt_t = [xtp.tile([128, S], bf16, tag="xt", name=f"xt{kt}")
                            for kt in range(KT)]
                    half = S // 2
                    for hh in range(2):
                        for kt in range(KT):
                            nc.sync.dma_start(
                                out=xt_t[kt][:, hh * half:(hh + 1) * half],
                                in_=xt[b, kt * 128:(kt + 1) * 128,
                                       hh * half:(hh + 1) * half])
                    st[b]["xt"] = xt_t
                    pen_sb = work.tile([128, NCHUNK], f32, tag="pen", name="pen")
                    nc.sync.dma_start(out=pen_sb, in_=pen[b])
                    st[b]["pen"] = pen_sb

            def project(xt_t, w_sb, bias_sb, dst, spans=None):
                split = isinstance(dst, list)
                for sp in (spans if spans is not None else range(NSPAN)):
                    ps = ps_po.tile([128, 512], f32, tag="po", name="ps")
                    for kt in range(KT):
                        nc.tensor.matmul(
                            ps,
                            w_sb[:, kt, :],
                            xt_t[kt][:, sp * 512:(sp + 1) * 512],
                            start=(kt == 0),
                            stop=(kt == KT - 1),
                        )
                    if split:
                        for h in range(HPC):
                            nc.vector.tensor_scalar_add(
                                out=dst[h][:, sp * 512:(sp + 1) * 512],
                                in0=ps[h * 64:(h + 1) * 64, :],
                                scalar1=bias_sb[h * 64:(h + 1) * 64, 0:1])
                    else:
                        nc.vector.tensor_scalar_add(
                            out=dst[:, sp * 512:(sp + 1) * 512],
                            in0=ps, scalar1=bias_sb[:, 0:1])

            def phase_proj_k(b, spans=None):
                if "ktp" not in st[b]:
                    st[b]["ktp"] = [
                        persist.tile([64, S], bf16, tag=f"ktp{h}",
                                     name=f"ktp{h}")
                        for h in range(HPC)]
                with nc.named_scope(f"projk{b}"):
                    project(st[b]["xt"], wk_sb, bk_sb, st[b]["ktp"],
                            spans=spans)

            def phase_proj_q(b, spans=None):
                if "qtp" not in st[b]:
                    st[b]["qtp"] = [
                        persist.tile([64, S], bf16, tag=f"qtp{h}",
                                     name=f"qtp{h}")
                        for h in range(HPC)]
                with nc.named_scope(f"projq{b}"):
                    project(st[b]["xt"], wq_sb, bq_sb, st[b]["qtp"],
                            spans=spans)

            def phase_proj_v(b, spans, chunks):
                with nc.named_scope(f"projv{b}"):
                    if "vt" not in st[b]:
                        st[b]["vt"] = persist.tile([128, S], bf16,
                                                   tag="vt", name="vt")
                        st[b]["vp"] = [
                            persist.tile([128, NCHUNK, 65], bf16,
                                         tag=f"vp{h}", name=f"vp{h}")
                            for h in range(HPC)]
                        for h in range(HPC):
                            nc.vector.memset(st[b]["vp"][h][:, :, 64:65], 1.0)
                    vt, vp = st[b]["vt"], st[b]["vp"]
                    project(st[b]["xt"], wv_sb, bv_sb, vt, spans=spans)
                    for c in chunks:
                        pt = ps_po.tile([128, 512], bf16, tag="po", name="pt")
                        nc.tensor.transpose(
                            pt[:, 0:128], vt[:, c * 128:(c + 1) * 128], ident)
                        for h in range(HPC):
                            nc.vector.tensor_copy(
                                out=vp[h][:, c, 0:64],
                                in_=pt[:, h * 64:(h + 1) * 64])

            def s_block(b, h, spp, half):
                """Scores matmuls + exp for 8 key chunks of one span pair."""
                qtp, ktp = st[b]["qtp"], st[b]["ktp"]
                pen_sb = st[b]["pen"]
                es = []
                with nc.named_scope(f"sS{b}{h}{spp}{half}"):
                    for kt in range(8 * half, 8 * half + 8):
                        sc = ps_sc.tile([128, 1024], f32, tag="sc", name="sc")
                        for i in range(2):
                            sp = 2 * spp + i
                            nc.tensor.matmul(
                                sc[:, i * 512:(i + 1) * 512],
                                ktp[h][:, kt * 128:(kt + 1) * 128],
                                qtp[h][:, sp * 512:(sp + 1) * 512],
                                start=True, stop=True,
                            )
                        e = epool.tile([128, 1024], bf16, tag="e", name="e")
                        nc.scalar.activation(e, sc, Exp, scale=0.125,
                                             bias=pen_sb[:, kt:kt + 1])
                        es.append(e)
                st[b][("es", h, spp, half)] = es

            def p_block(b, h, spp, half):
                """PV accumulation for 8 key chunks; drain on final half."""
                vp = st[b]["vp"]
                es = st[b].pop(("es", h, spp, half))
                if spp == 0 and half == 0:
                    st[b][("ctxt", h)] = ctxp.tile(
                        [65, S], f32, tag="ctxt", name=f"ctxt{h}")
                ctxt = st[b][("ctxt", h)]
                if half == 0:
                    st[b][("pc", h, spp)] = [
                        ps_pc.tile([65, 512], f32, tag="pc", name=f"pc{i}")
                        for i in range(2)]
                pcs = st[b][("pc", h, spp)]
                with nc.named_scope(f"sP{b}{h}{spp}{half}"):
                    for k, kt in enumerate(range(8 * half, 8 * half + 8)):
                        for i in range(2):
                            nc.tensor.matmul(
                                pcs[i], vp[h][:, kt, :],
                                es[k][:, i * 512:(i + 1) * 512],
                                start=(kt == 0), stop=(kt == NCHUNK - 1))
                    if half == 1:
                        for i in range(2):
                            nc.vector.tensor_copy(
                                out=ctxt[:, (2 * spp + i) * 512:
                                         (2 * spp + i + 1) * 512],
                                in_=pcs[i])
                if half == 1:
                    st[b].pop(("pc", h, spp))

            def phase_norm(b, h, spp):
                """Normalize one span pair of ctxt into ctxtn (bf16):
                reciprocal of the denominator row, gpsimd broadcast, one
                multiply — no DMA transposes."""
                ctxt = st[b][("ctxt", h)]
                if h == 0 and spp == 0:
                    st[b]["ctxtn"] = persist.tile(
                        [128, S], bf16, tag="ctxtn", name="ctxtn", bufs=1)
                ctxtn = st[b]["ctxtn"]
                o = spp * 1024
                with nc.named_scope(f"norm{b}{h}{spp}"):
                    rec = work.tile([1, 1024], f32, tag="rec", name="rec")
                    nc.vector.reciprocal(rec, ctxt[64:65, o:o + 1024])
                    rt = work.tile([64, 1024], f32, tag="rt", name="rt")
                    nc.gpsimd.partition_broadcast(out_ap=rt, in_ap=rec)
                    nc.vector.tensor_mul(
                        out=ctxtn[h * 64:(h + 1) * 64, o:o + 1024],
                        in0=ctxt[0:64, o:o + 1024], in1=rt)

            def phase_outproj(b, chunks, split_dma=False, scalar_cast=True,
                              deep_psum=False):
                ctxtn = st[b]["ctxtn"]
                with nc.named_scope(f"outproj{b}"):
                    for ci, c in enumerate(chunks):
                        ob = work.tile([128, DIM], bf16, tag="ob", name="ob")
                        for osp in range(2):
                            # deep_psum (tail only): borrow the dead scores
                            # pool to rotate 4 PSUM tiles so the matmul pairs
                            # never wait on the cast drains
                            if deep_psum and (ci * 2 + osp) % 4 >= 2:
                                po = ps_sc.tile([128, 512], f32, tag="sc",
                                                name="po2")
                            else:
                                po = ps_po.tile([128, 512], f32, tag="po",
                                                name="po")
                            nc.tensor.matmul(
                                po,
                                ctxtn[:, c * 128:(c + 1) * 128],
                                wo_sb[:, osp * 512:(osp + 1) * 512],
                                start=True, stop=True,
                            )
                            # alternate the PSUM drain between ScalarE and
                            # DVE: both have ~1.5us/slot of slack and the
                            # casts are the outproj bottleneck
                            if scalar_cast and osp == 1:
                                nc.scalar.activation(
                                    ob[:, osp * 512:(osp + 1) * 512], po, Copy)
                            else:
                                nc.vector.tensor_copy(
                                    out=ob[:, osp * 512:(osp + 1) * 512],
                                    in_=po)
                        if split_dma:
                            # split the store across two DMA rings to halve
                            # the per-queue drain at the very end
                            for dh in range(2):
                                nc.sync.dma_start(
                                    out=out[b, c * 128:(c + 1) * 128,
                                            dh * 512:(dh + 1) * 512],
                                    in_=ob[:, dh * 512:(dh + 1) * 512])
                        else:
                            nc.sync.dma_start(
                                out=out[b, c * 128:(c + 1) * 128, :], in_=ob)

            # ---------------- pipelined emission ----------------
            # units in (b, h, spp, half) order; S(j) ; P(j-1) ; side(j)
            units = [(b, h, spp, half)
                     for b in range(B) for h in range(HPC)
                     for spp in range(NSPP) for half in range(2)]

            phase_load(0)
            phase_proj_k(0, spans=(0, 1))
            phase_proj_q(0, spans=(0, 1))

            side = {
                1: lambda: (phase_proj_q(0, spans=(2, 3)),),
                2: lambda: (phase_load(1),),
                3: lambda: (phase_proj_k(1, spans=(0, 1, 2, 3)),),
                5: lambda: (phase_proj_v(1, spans=(0, 1, 2, 3),
                                         chunks=range(NCHUNK)),),
                7: lambda: (phase_proj_q(1, spans=(0, 1)),
                            phase_norm(0, 0, 0), phase_norm(0, 0, 1)),
                9: lambda: (phase_proj_q(1, spans=(2, 3)),
                            phase_norm(0, 1, 0), phase_norm(0, 1, 1)),
                11: lambda: (phase_outproj(0, chunks=range(8),
                                           scalar_cast=False),),
                12: lambda: (phase_outproj(0, chunks=range(8, 16),
                                           scalar_cast=False),),
                13: lambda: (phase_norm(1, 0, 0),),
                14: lambda: (phase_norm(1, 0, 1),),
                15: lambda: (phase_norm(1, 1, 0),),
            }

            s_block(*units[0])
            phase_proj_k(0, spans=(2, 3))
            phase_proj_v(0, spans=(0, 1, 2, 3), chunks=range(NCHUNK))
            for j in range(1, len(units)):
                s_block(*units[j])
                p_block(*units[j - 1])
                if j in side:
                    side[j]()
            p_block(*units[-1])
            phase_norm(1, 1, 1)
            phase_outproj(1, chunks=range(NCHUNK), scalar_cast=True,
                          split_dma=True, deep_psum=True)

    nc.compile()
    return nc


def _get_nc():
    if "nc" not in _cached:
        _cached["nc"] = _build()
    return _cached["nc"]


def _prep_inputs(X, mask, Wq, bq, Wk, bk, Wv, bv, Wo, bo):
    import ml_dtypes
    bf16 = ml_dtypes.bfloat16

    X = np.asarray(X, dtype=np.float32)
    mask = np.asarray(mask, dtype=np.float32)
    Wq, Wk, Wv, Wo = (np.asarray(a, dtype=np.float32) for a in (Wq, Wk, Wv, Wo))
    bq, bk, bv = (np.asarray(a, dtype=np.float32) for a in (bq, bk, bv))

    xtf = np.ascontiguousarray(X.transpose(0, 2, 1)).astype(bf16)  # [B, DIM, S]
    pen_full = (-1e6 * (1.0 - mask)).astype(np.float32)      # [B, S]
    # bias tile layout: pen_sb[p, kt] = pen_full[b, kt*128 + p]
    penf = np.ascontiguousarray(
        pen_full.reshape(B, NCHUNK, 128).transpose(0, 2, 1))

    in_maps = []
    for c in range(N_CORES):
        sl = slice(c * DHC, (c + 1) * DHC)
        in_maps.append({
            "xt": xtf,
            "pen": penf,
            "wq": np.ascontiguousarray(Wq[:, sl]).astype(bf16),
            "wk": np.ascontiguousarray(Wk[:, sl]).astype(bf16),
            "wv": np.ascontiguousarray(Wv[:, sl]).astype(bf16),
            "wo": np.ascontiguousarray(Wo[sl, :]).astype(bf16),
            "bq": np.ascontiguousarray(bq[sl].reshape(DHC, 1)),
            "bk": np.ascontiguousarray(bk[sl].reshape(DHC, 1)),
            "bv": np.ascontiguousarray(bv[sl].reshape(DHC, 1)),
        })
    return in_maps


def kernel(X, mask, Wq, bq, Wk, bk, Wv, bv, Wo, bo):
    from concourse.bass_utils import run_bass_kernel_spmd

    in_maps = _prep_inputs(X, mask, Wq, bq, Wk, bk, Wv, bv, Wo, bo)
    res = run_bass_kernel_spmd(_get_nc(), in_maps, core_ids=list(range(N_CORES)))
    _cached["last_results"] = res
    acc = res.results[0]["out"].astype(np.float32).copy()
    for c in range(1, N_CORES):
        acc += res.results[c]["out"].astype(np.float32)
    acc += np.asarray(bo, dtype=np.float32)[None, None, :]
    return acc.astype(np.float32)
